# revision 5
# baseline (speedup 1.0000x reference)
"""Multi-head attention TRN2 Bass kernel.

Problem: B=2, S=2048, D_MODEL=1024, N_HEAD=16, D_HEAD=64.
  qh = split_heads(q @ Wq.T + bq) / 8;  kh, vh likewise (no scale)
  scores = (qh @ kh.T) * mask;  attn = softmax(scores);  ctx = attn @ vh
  out = ctx.reshape(B, S, 1024)   (raw [b,h,s,d] view, no head transpose-back)

Sharding (8 cores, tensor-parallel over heads + data-parallel over batch):
  core c -> batch b = c//4, heads [4*(c%4), 4*(c%4)+4)  (256 feature cols).

Per-core dataflow (all matmuls bf16 inputs, f32 PSUM accumulation):
  - host feeds x.T (bf16), W.T slices (bf16), mask.T (f32) so no on-chip
    transposes are needed anywhere.
  - Q.T/K.T computed in [dh, s] layout (activation adds bias + casts bf16,
    Q pre-scaled by 1/8 via activation scale with pre-scaled host bias).
  - V computed in natural [s, f] layout with a ones column appended per head
    so the attn@V matmul also emits the softmax denominator (column 64).
  - scores.T tiles [sk=128, sq] via PE (K=64); mask multiply on DVE from
    PSUM into f32 staging; Exp on ACT into bf16 probs; attn@[V|1] accumulates
    over sk tiles in PSUM; normalize by reciprocal of column 64.
"""

import numpy as np
import ml_dtypes
from contextlib import ExitStack

import concourse.bass as bass
import concourse.mybir as mybir
import concourse.tile as tile
from concourse import bacc
from concourse.bass_utils import run_bass_kernel_spmd

BF16 = ml_dtypes.bfloat16

B = 2
D_MODEL = 1024
N_HEAD = 16
D_HEAD = 64
N_CORES = 8
HEADS_PER_CORE = 4
F = HEADS_PER_CORE * D_HEAD  # 256 feature cols per core
KC = D_MODEL // 128  # 8 contraction chunks

_NC_CACHE = {}


def build_nc(S=2048):
    f32 = mybir.dt.float32
    bf16 = mybir.dt.bfloat16
    Ident = mybir.ActivationFunctionType.Identity
    Exp = mybir.ActivationFunctionType.Exp

    SQC = min(512, S)        # sq chunk (one scores matmul N)
    NSC = S // SQC           # number of sq chunks
    NT = S // 128            # number of sk tiles
    EG = 4                   # sk tiles per Exp batch
    NEG = (NT + EG - 1) // EG

    nc = bacc.Bacc("TRN2", target_bir_lowering=False, debug=False)

    xqT = nc.dram_tensor("xqT", [D_MODEL, S], bf16, kind="ExternalInput").ap()
    xkT = nc.dram_tensor("xkT", [D_MODEL, S], bf16, kind="ExternalInput").ap()
    xvT = nc.dram_tensor("xvT", [D_MODEL, S], bf16, kind="ExternalInput").ap()
    wqT = nc.dram_tensor("wqT", [D_MODEL, F], bf16, kind="ExternalInput").ap()
    wkT = nc.dram_tensor("wkT", [D_MODEL, F], bf16, kind="ExternalInput").ap()
    wvT = nc.dram_tensor("wvT", [D_MODEL, F], bf16, kind="ExternalInput").ap()
    bqs = nc.dram_tensor("bqs", [128, 2], f32, kind="ExternalInput").ap()
    bks = nc.dram_tensor("bks", [128, 2], f32, kind="ExternalInput").ap()
    bvr = nc.dram_tensor("bvr", [1, F], f32, kind="ExternalInput").ap()
    maskT = nc.dram_tensor("maskT", [S, S], f32, kind="ExternalInput").ap()
    out = nc.dram_tensor("out", [S, F], f32, kind="ExternalOutput").ap()

    xqT_r = xqT.rearrange("(c p) s -> p c s", p=128)
    xkT_r = xkT.rearrange("(c p) s -> p c s", p=128)
    xvT_r = xvT.rearrange("(c p) s -> p c s", p=128)
    maskT_r = maskT.rearrange("(t p) q -> p t q", p=128)

    with tile.TileContext(nc) as tc, ExitStack() as ctx:
        const = ctx.enter_context(tc.tile_pool(name="const", bufs=1))
        persist = ctx.enter_context(tc.tile_pool(name="persist", bufs=1))
        mpool = ctx.enter_context(tc.tile_pool(name="mpool", bufs=2))
        fpool = ctx.enter_context(tc.tile_pool(name="fpool", bufs=2))
        ppool = ctx.enter_context(tc.tile_pool(name="ppool", bufs=2))
        opool = ctx.enter_context(tc.tile_pool(name="opool", bufs=3))
        tiny = ctx.enter_context(tc.tile_pool(name="tiny", bufs=4))
        proj_ctx = ctx.enter_context(ExitStack())
        xpool = proj_ctx.enter_context(tc.tile_pool(name="xpool", bufs=2))
        psA = proj_ctx.enter_context(tc.tile_pool(name="psA", bufs=2, space="PSUM"))
        psB = proj_ctx.enter_context(tc.tile_pool(name="psB", bufs=2, space="PSUM"))

        # --- constants -----------------------------------------------------
        wq_sb = const.tile([128, KC, F], bf16)
        nc.sync.dma_start(wq_sb[:], wqT.rearrange("(c p) f -> p c f", p=128))
        wk_sb = const.tile([128, KC, F], bf16)
        nc.sync.dma_start(wk_sb[:], wkT.rearrange("(c p) f -> p c f", p=128))
        wv_sb = const.tile([128, KC, F], bf16)
        nc.sync.dma_start(wv_sb[:], wvT.rearrange("(c p) f -> p c f", p=128))
        bq_sb = const.tile([128, 2], f32)
        nc.sync.dma_start(bq_sb[:], bqs)
        bk_sb = const.tile([128, 2], f32)
        nc.sync.dma_start(bk_sb[:], bks)
        bv_sb = const.tile([1, F], f32)
        nc.sync.dma_start(bv_sb[:], bvr)

        ones1 = const.tile([1, 128], f32)
        nc.vector.memset(ones1[:], 1.0)
        bvp = psB.tile([128, F], f32, tag="psv")
        nc.tensor.matmul(bvp[:], ones1[:], bv_sb[:], start=True, stop=True)
        bvRep = const.tile([128, F], f32)
        nc.vector.tensor_copy(bvRep[:], bvp[:])

        QT_sb = persist.tile([128, 2, S], bf16)
        KT_sb = persist.tile([128, 2, S], bf16)
        V_sb = persist.tile([128, NT, HEADS_PER_CORE * (D_HEAD + 1)], bf16)
        # ones columns (denominator trick): col 64 of each head's 65-col group
        V4 = V_sb.rearrange("p t (h j) -> p t h j", h=HEADS_PER_CORE)
        nc.vector.memset(V4[:, :, :, D_HEAD : D_HEAD + 1], 1.0)

        # --- projections ---------------------------------------------------
        # K first, then V, then Q (attention chunk 0 needs all of K and V).
        for sc in range(NSC):
            xk_t = xpool.tile([128, KC, SQC], bf16, tag="xt")
            nc.sync.dma_start(xk_t[:], xkT_r[:, :, sc * SQC : (sc + 1) * SQC])
            for ft in range(2):
                ps = psA.tile([128, SQC], f32, tag="psproj")
                for kc in range(KC):
                    nc.tensor.matmul(
                        ps[:],
                        wk_sb[:, kc, ft * 128 : (ft + 1) * 128],
                        xk_t[:, kc, :],
                        start=(kc == 0),
                        stop=(kc == KC - 1),
                    )
                # bias add (per-partition) + cast bf16, on DVE
                nc.vector.tensor_scalar_add(
                    KT_sb[:, ft, sc * SQC : (sc + 1) * SQC], ps[:], bk_sb[:, ft : ft + 1]
                )

        for sc in range(NSC):
            xv_t = xpool.tile([128, KC, SQC], bf16, tag="xt")
            nc.sync.dma_start(xv_t[:], xvT_r[:, :, sc * SQC : (sc + 1) * SQC])
            for st in range(SQC // 128):
                t = sc * (SQC // 128) + st
                ps = psB.tile([128, F], f32, tag="psv")
                for kc in range(KC):
                    nc.tensor.matmul(
                        ps[:],
                        xv_t[:, kc, st * 128 : (st + 1) * 128],
                        wv_sb[:, kc, :],
                        start=(kc == 0),
                        stop=(kc == KC - 1),
                    )
                nc.vector.tensor_add(
                    V4[:, t, :, 0:D_HEAD],
                    ps[:].rearrange("p (h j) -> p h j", h=HEADS_PER_CORE),
                    bvRep.rearrange("p (h j) -> p h j", h=HEADS_PER_CORE),
                )

        for sc in range(NSC):
            xq_t = xpool.tile([128, KC, SQC], bf16, tag="xt")
            nc.sync.dma_start(xq_t[:], xqT_r[:, :, sc * SQC : (sc + 1) * SQC])
            for ft in range(2):
                ps = psA.tile([128, SQC], f32, tag="psproj")
                for kc in range(KC):
                    nc.tensor.matmul(
                        ps[:],
                        wq_sb[:, kc, ft * 128 : (ft + 1) * 128],
                        xq_t[:, kc, :],
                        start=(kc == 0),
                        stop=(kc == KC - 1),
                    )
                # out = in * 0.125 + bq (bq pre-scaled on host); cast bf16; ACT
                nc.scalar.activation(
                    QT_sb[:, ft, sc * SQC : (sc + 1) * SQC],
                    ps[:],
                    Ident,
                    bias=bq_sb[:, ft : ft + 1],
                    scale=0.125,
                )

        # --- attention -----------------------------------------------------
        proj_ctx.close()  # release xpool/psA/psB (PSUM) for the attention pools
        psC = ctx.enter_context(tc.tile_pool(name="psC", bufs=4, space="PSUM"))
        psD = ctx.enter_context(tc.tile_pool(name="psD", bufs=3, space="PSUM"))
        for sc in range(NSC):
            m_t = mpool.tile([128, NT, SQC], f32)
            nc.sync.dma_start(m_t[:], maskT_r[:, :, sc * SQC : (sc + 1) * SQC])
            for h in range(HEADS_PER_CORE):
                fr = (h % 2) * 64
                ft = h // 2
                probs = ppool.tile([128, NT, SQC], bf16)
                for g in range(NEG):
                    t0 = g * EG
                    t1 = min(NT, t0 + EG)
                    pf = fpool.tile([128, EG, SQC], f32, tag="pf")
                    for t in range(t0, t1):
                        ps = psC.tile([128, SQC], f32, tag="pss")
                        nc.tensor.matmul(
                            ps[:],
                            KT_sb[fr : fr + 64, ft, t * 128 : (t + 1) * 128],
                            QT_sb[fr : fr + 64, ft, sc * SQC : (sc + 1) * SQC],
                            start=True,
                            stop=True,
                        )
                        nc.vector.tensor_mul(pf[:, t - t0, :], ps[:], m_t[:, t, :])
                    nc.scalar.activation(
                        probs[:, t0:t1, :], pf[:, 0 : t1 - t0, :], Exp
                    )
                for st in range(SQC // 128):
                    pc = psD.tile([128, D_HEAD + 1], f32, tag="psc")
                    for t in range(NT):
                        nc.tensor.matmul(
                            pc[:],
                            probs[:, t, st * 128 : (st + 1) * 128],
                            V_sb[:, t, h * (D_HEAD + 1) : (h + 1) * (D_HEAD + 1)],
                            start=(t == 0),
                            stop=(t == NT - 1),
                        )
                    rec = tiny.tile([128, 1], f32)
                    nc.vector.reciprocal(rec[:], pc[:, D_HEAD : D_HEAD + 1])
                    ot = opool.tile([128, D_HEAD], f32)
                    nc.vector.tensor_scalar_mul(ot[:], pc[:, 0:D_HEAD], rec[:])
                    r0 = sc * SQC + st * 128
                    nc.sync.dma_start(
                        out[r0 : r0 + 128, h * D_HEAD : (h + 1) * D_HEAD], ot[:]
                    )

    nc.compile()
    return nc


def make_in_maps(q, k, v, mask, Wq, bq, Wk, bk, Wv, bv):
    """Build the 8 per-core input maps (host-side shard + transpose + cast)."""
    S = q.shape[1]
    xT = {}
    mT = {}
    for b in range(B):
        xT[("q", b)] = np.ascontiguousarray(q[b].T).astype(BF16)
        xT[("k", b)] = np.ascontiguousarray(k[b].T).astype(BF16)
        xT[("v", b)] = np.ascontiguousarray(v[b].T).astype(BF16)
        mT[b] = np.ascontiguousarray(mask[b, 0].T)

    in_maps = []
    for c in range(N_CORES):
        b = c // 4
        g = c % 4
        fs = slice(g * F, (g + 1) * F)
        in_maps.append(
            {
                "xqT": xT[("q", b)],
                "xkT": xT[("k", b)],
                "xvT": xT[("v", b)],
                "wqT": np.ascontiguousarray(Wq[fs].T).astype(BF16),
                "wkT": np.ascontiguousarray(Wk[fs].T).astype(BF16),
                "wvT": np.ascontiguousarray(Wv[fs].T).astype(BF16),
                "bqs": np.ascontiguousarray(
                    (bq[fs] * 0.125).reshape(2, 128).T
                ).astype(np.float32),
                "bks": np.ascontiguousarray(bk[fs].reshape(2, 128).T).astype(
                    np.float32
                ),
                "bvr": np.ascontiguousarray(bv[fs].reshape(1, F)).astype(np.float32),
                "maskT": mT[b],
            }
        )
    return in_maps


def assemble_output(results, S=2048):
    ctx_all = np.empty((B, N_HEAD, S, D_HEAD), np.float32)
    for c in range(N_CORES):
        b = c // 4
        g = c % 4
        oc = results[c]["out"]  # [S, F]
        ctx_all[b, g * 4 : (g + 1) * 4] = oc.reshape(S, HEADS_PER_CORE, D_HEAD).transpose(
            1, 0, 2
        )
    return ctx_all.reshape(B, S, N_HEAD * D_HEAD)


def run_cores(in_maps, trace=False, **kwargs):
    if "nc" not in _NC_CACHE:
        _NC_CACHE["nc"] = build_nc()
    nc = _NC_CACHE["nc"]
    return run_bass_kernel_spmd(
        nc, in_maps, core_ids=list(range(N_CORES)), trace=trace, **kwargs
    )


def kernel(q, k, v, mask, Wq, bq, Wk, bk, Wv, bv):
    q = np.asarray(q, dtype=np.float32)
    k = np.asarray(k, dtype=np.float32)
    v = np.asarray(v, dtype=np.float32)
    mask = np.asarray(mask, dtype=np.float32)
    in_maps = make_in_maps(
        q,
        k,
        v,
        mask,
        np.asarray(Wq, np.float32),
        np.asarray(bq, np.float32),
        np.asarray(Wk, np.float32),
        np.asarray(bk, np.float32),
        np.asarray(Wv, np.float32),
        np.asarray(bv, np.float32),
    )
    res = run_cores(in_maps)
    return assemble_output(res.results, S=q.shape[1])


# revision 6
# speedup vs baseline: 1.2594x; 1.2594x over previous
"""Multi-head attention TRN2 Bass kernel.

Problem: B=2, S=2048, D_MODEL=1024, N_HEAD=16, D_HEAD=64.
  qh = split_heads(q @ Wq.T + bq) / 8;  kh, vh likewise (no scale)
  scores = (qh @ kh.T) * mask;  attn = softmax(scores);  ctx = attn @ vh
  out = ctx.reshape(B, S, 1024)   (raw [b,h,s,d] view, no head transpose-back)

Sharding (8 cores, tensor-parallel over heads + data-parallel over batch):
  core c -> batch b = c//4, heads [4*(c%4), 4*(c%4)+4)  (256 feature cols).

Per-core dataflow (all matmuls bf16 inputs, f32 PSUM accumulation):
  - host feeds x.T (bf16), W.T slices (bf16), mask.T (f32): no on-chip
    transposes needed anywhere.
  - Q.T in [dh, s] layout packed per head-pair; K.T per head zero-padded to
    128 contraction rows (K=64 matmuls stream the moving operand at half
    rate, so padding the stationary with zeros doubles throughput).
  - V in natural [s, f] layout with a ones column per head so the attn@V
    matmul also emits the softmax denominator.
  - scores.T tiles [sk=128, sq=512] via PE; mask multiply on DVE from PSUM
    into f32 staging; Exp on ACT into bf16 probs; ctx.T = (V|1).T @ probs.T
    accumulates over sk tiles with V|1 stationary (no per-tile weight
    reloads); PSUM->SBUF on ACT; normalization (divide by ones-column row)
    happens on the host after gather.
"""

import numpy as np
import ml_dtypes
from contextlib import ExitStack

import concourse.bass as bass
import concourse.mybir as mybir
import concourse.tile as tile
from concourse import bacc
from concourse.bass_utils import run_bass_kernel_spmd

BF16 = ml_dtypes.bfloat16

B = 2
D_MODEL = 1024
N_HEAD = 16
D_HEAD = 64
N_CORES = 8
HEADS_PER_CORE = 4
F = HEADS_PER_CORE * D_HEAD  # 256 feature cols per core
KC = D_MODEL // 128  # 8 contraction chunks

_NC_CACHE = {}


def build_nc(S=2048):
    f32 = mybir.dt.float32
    bf16 = mybir.dt.bfloat16
    Ident = mybir.ActivationFunctionType.Identity
    Exp = mybir.ActivationFunctionType.Exp

    SQC = min(512, S)        # sq chunk (one scores matmul N)
    NSC = S // SQC           # number of sq chunks
    NT = S // 128            # number of sk tiles
    EG = 4                   # sk tiles per Exp batch
    NEG = (NT + EG - 1) // EG
    D1 = D_HEAD + 1          # head cols incl. denominator ones column

    nc = bacc.Bacc("TRN2", target_bir_lowering=False, debug=False)

    xqT = nc.dram_tensor("xqT", [D_MODEL, S], bf16, kind="ExternalInput").ap()
    xkT = nc.dram_tensor("xkT", [D_MODEL, S], bf16, kind="ExternalInput").ap()
    xvT = nc.dram_tensor("xvT", [D_MODEL, S], bf16, kind="ExternalInput").ap()
    wqT = nc.dram_tensor("wqT", [D_MODEL, F], bf16, kind="ExternalInput").ap()
    wkT = nc.dram_tensor("wkT", [D_MODEL, F], bf16, kind="ExternalInput").ap()
    wvT = nc.dram_tensor("wvT", [D_MODEL, F], bf16, kind="ExternalInput").ap()
    bqs = nc.dram_tensor("bqs", [128, 2], f32, kind="ExternalInput").ap()
    bks = nc.dram_tensor("bks", [128, 2], f32, kind="ExternalInput").ap()
    bvr = nc.dram_tensor("bvr", [1, F], f32, kind="ExternalInput").ap()
    maskT = nc.dram_tensor("maskT", [S, S], f32, kind="ExternalInput").ap()
    # per-head transposed context incl. denominator row (normalized on host)
    out = nc.dram_tensor(
        "out", [HEADS_PER_CORE, D1, S], f32, kind="ExternalOutput"
    ).ap()

    xqT_r = xqT.rearrange("(c p) s -> p c s", p=128)
    xkT_r = xkT.rearrange("(c p) s -> p c s", p=128)
    xvT_r = xvT.rearrange("(c p) s -> p c s", p=128)
    maskT_r = maskT.rearrange("(t p) q -> p t q", p=128)

    with tile.TileContext(nc) as tc, ExitStack() as ctx:
        const = ctx.enter_context(tc.tile_pool(name="const", bufs=1))
        persist = ctx.enter_context(tc.tile_pool(name="persist", bufs=1))
        xpool = ctx.enter_context(tc.tile_pool(name="xpool", bufs=2))
        mpool = ctx.enter_context(tc.tile_pool(name="mpool", bufs=2))
        fpool = ctx.enter_context(tc.tile_pool(name="fpool", bufs=2))
        ppool = ctx.enter_context(tc.tile_pool(name="ppool", bufs=2))
        copool = ctx.enter_context(tc.tile_pool(name="copool", bufs=3))
        # PSUM: 3 (proj) + 3 (scores) + 2 (ctx) = 8 banks
        psP = ctx.enter_context(tc.tile_pool(name="psP", bufs=3, space="PSUM"))
        psC = ctx.enter_context(tc.tile_pool(name="psC", bufs=3, space="PSUM"))
        psD = ctx.enter_context(tc.tile_pool(name="psD", bufs=2, space="PSUM"))

        # --- constants -----------------------------------------------------
        wq_sb = const.tile([128, KC, F], bf16)
        nc.sync.dma_start(wq_sb[:], wqT.rearrange("(c p) f -> p c f", p=128))
        wk_sb = const.tile([128, KC, F], bf16)
        nc.sync.dma_start(wk_sb[:], wkT.rearrange("(c p) f -> p c f", p=128))
        wv_sb = const.tile([128, KC, F], bf16)
        nc.sync.dma_start(wv_sb[:], wvT.rearrange("(c p) f -> p c f", p=128))
        bq_sb = const.tile([128, 2], f32)
        nc.sync.dma_start(bq_sb[:], bqs)
        bk_sb = const.tile([128, 2], f32)
        nc.sync.dma_start(bk_sb[:], bks)
        bv_sb = const.tile([1, F], f32)
        nc.sync.dma_start(bv_sb[:], bvr)

        ones1 = const.tile([1, 128], f32)
        nc.vector.memset(ones1[:], 1.0)
        bvp = psP.tile([128, F], f32, tag="pp")
        nc.tensor.matmul(bvp[:], ones1[:], bv_sb[:], start=True, stop=True)
        bvRep = const.tile([128, F], f32)
        nc.vector.tensor_copy(bvRep[:], bvp[:])

        QT_sb = persist.tile([128, 2, S], bf16)
        # K.T zero-padded per head: head h lives in rows (h%2)*64..+64 of
        # KTz[:, h, :], the other 64 rows stay zero -> scores run at K=128.
        KTz = persist.tile([128, HEADS_PER_CORE, S], bf16)
        for h in range(HEADS_PER_CORE):
            zr = (1 - h % 2) * 64
            nc.vector.memset(KTz[zr : zr + 64, h, :], 0.0)
        V_sb = persist.tile([128, NT, HEADS_PER_CORE * D1], bf16)
        V4 = V_sb.rearrange("p t (h j) -> p t h j", h=HEADS_PER_CORE)
        nc.vector.memset(V4[:, :, :, D_HEAD : D_HEAD + 1], 1.0)

        # --- K projection (+ head-split into KTz halves, bias on ACT) ------
        for sc in range(NSC):
            xk_t = xpool.tile([128, KC, SQC], bf16, tag="xt")
            nc.sync.dma_start(xk_t[:], xkT_r[:, :, sc * SQC : (sc + 1) * SQC])
            for ft in range(2):
                ps = psP.tile([128, SQC], f32, tag="pp")
                for kc in range(KC):
                    nc.tensor.matmul(
                        ps[:],
                        wk_sb[:, kc, ft * 128 : (ft + 1) * 128],
                        xk_t[:, kc, :],
                        start=(kc == 0),
                        stop=(kc == KC - 1),
                    )
                sl = slice(sc * SQC, (sc + 1) * SQC)
                nc.scalar.activation(
                    KTz[0:64, 2 * ft, sl], ps[0:64, :], Ident, bias=bk_sb[0:64, ft : ft + 1]
                )
                nc.scalar.activation(
                    KTz[64:128, 2 * ft + 1, sl],
                    ps[64:128, :],
                    Ident,
                    bias=bk_sb[64:128, ft : ft + 1],
                )

        # --- Q projection chunk maker (bias+scale on ACT) ------------------
        def q_proj(sc):
            xq_t = xpool.tile([128, KC, SQC], bf16, tag="xt", name=f"xq_{sc}")
            nc.sync.dma_start(xq_t[:], xqT_r[:, :, sc * SQC : (sc + 1) * SQC])
            for ft in range(2):
                ps = psP.tile([128, SQC], f32, tag="pp", name=f"psq_{sc}_{ft}")
                for kc in range(KC):
                    nc.tensor.matmul(
                        ps[:],
                        wq_sb[:, kc, ft * 128 : (ft + 1) * 128],
                        xq_t[:, kc, :],
                        start=(kc == 0),
                        stop=(kc == KC - 1),
                    )
                # out = in*0.125 + bq (bq pre-scaled by 0.125 on host)
                nc.scalar.activation(
                    QT_sb[:, ft, sc * SQC : (sc + 1) * SQC],
                    ps[:],
                    Ident,
                    bias=bq_sb[:, ft : ft + 1],
                    scale=0.125,
                )

        q_proj(0)

        # --- V projection (bias add on DVE via replicated-bias trick) ------
        for sc in range(NSC):
            xv_t = xpool.tile([128, KC, SQC], bf16, tag="xt")
            nc.sync.dma_start(xv_t[:], xvT_r[:, :, sc * SQC : (sc + 1) * SQC])
            for st in range(SQC // 128):
                t = sc * (SQC // 128) + st
                ps = psP.tile([128, F], f32, tag="pp")
                for kc in range(KC):
                    nc.tensor.matmul(
                        ps[:],
                        xv_t[:, kc, st * 128 : (st + 1) * 128],
                        wv_sb[:, kc, :],
                        start=(kc == 0),
                        stop=(kc == KC - 1),
                    )
                nc.vector.tensor_add(
                    V4[:, t, :, 0:D_HEAD],
                    ps[:].rearrange("p (h j) -> p h j", h=HEADS_PER_CORE),
                    bvRep.rearrange("p (h j) -> p h j", h=HEADS_PER_CORE),
                )

        # --- attention (interleave remaining Q chunks) ----------------------
        for sc in range(NSC):
            if sc > 0:
                q_proj(sc)
            m_t = mpool.tile([128, NT, SQC], f32)
            nc.sync.dma_start(m_t[:], maskT_r[:, :, sc * SQC : (sc + 1) * SQC])
            for h in range(HEADS_PER_CORE):
                ft = h // 2
                probs = ppool.tile([128, NT, SQC], bf16)
                for g in range(NEG):
                    t0 = g * EG
                    t1 = min(NT, t0 + EG)
                    pf = fpool.tile([128, EG, SQC], f32, tag="pf")
                    for t in range(t0, t1):
                        ps = psC.tile([128, SQC], f32, tag="pss")
                        nc.tensor.matmul(
                            ps[:],
                            KTz[:, h, t * 128 : (t + 1) * 128],
                            QT_sb[:, ft, sc * SQC : (sc + 1) * SQC],
                            start=True,
                            stop=True,
                        )
                        nc.vector.tensor_mul(pf[:, t - t0, :], ps[:], m_t[:, t, :])
                    nc.scalar.activation(probs[:, t0:t1, :], pf[:, 0 : t1 - t0, :], Exp)
                # ctx.T = (V|1).T @ probs.T : V|1 stationary, probs moving
                pc = psD.tile([D1, SQC], f32, tag="psc")
                for t in range(NT):
                    nc.tensor.matmul(
                        pc[:],
                        V_sb[:, t, h * D1 : (h + 1) * D1],
                        probs[:, t, :],
                        start=(t == 0),
                        stop=(t == NT - 1),
                    )
                co = copool.tile([D1, SQC], f32)
                nc.scalar.copy(co[:], pc[:])
                nc.sync.dma_start(out[h, :, sc * SQC : (sc + 1) * SQC], co[:])

    nc.compile()
    return nc


def make_in_maps(q, k, v, mask, Wq, bq, Wk, bk, Wv, bv):
    """Build the 8 per-core input maps (host-side shard + transpose + cast)."""
    xT = {}
    mT = {}
    for b in range(B):
        xT[("q", b)] = np.ascontiguousarray(q[b].T).astype(BF16)
        xT[("k", b)] = np.ascontiguousarray(k[b].T).astype(BF16)
        xT[("v", b)] = np.ascontiguousarray(v[b].T).astype(BF16)
        mT[b] = np.ascontiguousarray(mask[b, 0].T)

    in_maps = []
    for c in range(N_CORES):
        b = c // 4
        g = c % 4
        fs = slice(g * F, (g + 1) * F)
        in_maps.append(
            {
                "xqT": xT[("q", b)],
                "xkT": xT[("k", b)],
                "xvT": xT[("v", b)],
                "wqT": np.ascontiguousarray(Wq[fs].T).astype(BF16),
                "wkT": np.ascontiguousarray(Wk[fs].T).astype(BF16),
                "wvT": np.ascontiguousarray(Wv[fs].T).astype(BF16),
                "bqs": np.ascontiguousarray(
                    (bq[fs] * 0.125).reshape(2, 128).T
                ).astype(np.float32),
                "bks": np.ascontiguousarray(bk[fs].reshape(2, 128).T).astype(
                    np.float32
                ),
                "bvr": np.ascontiguousarray(bv[fs].reshape(1, F)).astype(np.float32),
                "maskT": mT[b],
            }
        )
    return in_maps


def assemble_output(results, S=2048):
    ctx_all = np.empty((B, N_HEAD, S, D_HEAD), np.float32)
    for c in range(N_CORES):
        b = c // 4
        g = c % 4
        oc = results[c]["out"]  # [4, 65, S]: rows 0..63 ctx.T, row 64 denom
        for hl in range(HEADS_PER_CORE):
            ctx_all[b, g * 4 + hl] = (oc[hl, :D_HEAD, :] / oc[hl, D_HEAD:, :]).T
    return ctx_all.reshape(B, S, N_HEAD * D_HEAD)


def run_cores(in_maps, trace=False, **kwargs):
    if "nc" not in _NC_CACHE:
        _NC_CACHE["nc"] = build_nc()
    nc = _NC_CACHE["nc"]
    return run_bass_kernel_spmd(
        nc, in_maps, core_ids=list(range(N_CORES)), trace=trace, **kwargs
    )


def kernel(q, k, v, mask, Wq, bq, Wk, bk, Wv, bv):
    q = np.asarray(q, dtype=np.float32)
    k = np.asarray(k, dtype=np.float32)
    v = np.asarray(v, dtype=np.float32)
    mask = np.asarray(mask, dtype=np.float32)
    in_maps = make_in_maps(
        q,
        k,
        v,
        mask,
        np.asarray(Wq, np.float32),
        np.asarray(bq, np.float32),
        np.asarray(Wk, np.float32),
        np.asarray(bk, np.float32),
        np.asarray(Wv, np.float32),
        np.asarray(bv, np.float32),
    )
    res = run_cores(in_maps)
    return assemble_output(res.results, S=q.shape[1])
